# revision 17
# baseline (speedup 1.0000x reference)
"""Multi-head attention (B=2, S=2048, D=1024, H=16, E=64) on 8 NeuronCores.

Sharding: core c = (batch b, head-group hg) with b = c // 4, hg = c % 4.
Each core projects q/k/v for its batch into its 4 heads, runs dense
attention for those heads over the full sequence, and computes a partial
output projection with its 256 rows of Wo.  The host sums the 4 partials
per batch and adds bo (the TP all-reduce, folded into the gather step).

v2 layout/schedule:
  - x loaded as 8 [128, 2048] tiles per tensor (24 big DMAs on sync).
  - K-proj, Q-proj j-waved over 4 [128,512] psum slots (pool A); V-proj
    cycles the same slots; later the slots serve PV accumulators, rb
    broadcasts and out-proj psums.  Scores get a dedicated 2x[128,1024]
    pool (B).  Together exactly 8 PSUM banks, no pool scoping barriers.
  - q/k biases folded into the psum evictions (per-partition bias AP on
    ACT/DVE), v bias via a K=1 ones matmul as before.
  - exp split between ACT (activation Exp) and DVE: a one-instruction
    Schraudolph exp2 - tensor_scalar(mult,add) -> int16, bitcast to bf16.
    (fp32->int16 conversion truncates, so +0.5 is folded into the bias.)
  - softmax denominators ride the ones-column of vh' (row 64 of the PV
    psum); per head-pair they land in sums_j via tiny DMAs, reciprocal by
    RECIPROCAL_APPROX_FAST, broadcast by an e-matrix matmul, applied to
    attnT in place; out-projection then runs per (st, nh) and is DMAed
    out as bf16 partials.
"""

import numpy as np

B, S, D, H, E = 2, 2048, 1024, 16, 64
HG = 4            # heads per core
N_CORES = 8
EL = E + 1        # 65: head block width in vh' (values + ones column)
DT = D // 128     # 8 contraction tiles
SC = S // 512     # 4 s-chunks of 512
TT = S // 128     # 16 key tiles

# Schraudolph exp(x/8) ~= bitcast_bf16(int16(x*EXP_A + EXP_B))
EXP_A = 16.0 * np.log2(np.e)                      # 128*log2(e)/8
EXP_B = 127.0 * 128 - 128 * np.log2(1.0302) + 0.5  # center + trunc->round
# which key-tiles go to the DVE exp path (7 of 16)
DVE_TT = frozenset((1, 3, 5, 7, 9, 11, 13, 15))

_NC = None        # cached compiled Bass module

# e_sb[k, sc*128 + m] = (k == (m//64)*4 + sc): broadcasts recip row
# (head_local*4 + sc) of a head-pair sums tile to output partition m.
_ESB = np.zeros((8, 4 * 128), np.float32)
for _sc in range(4):
    for _m in range(128):
        _ESB[(_m // 64) * 4 + _sc, _sc * 128 + _m] = 1.0
_ONES = np.ones((1, 512), np.float32)


def _build():
    import concourse.bass as bass
    import concourse.mybir as mybir
    import concourse.tile as tile
    from concourse import bacc

    FP = mybir.dt.float32
    FPR = mybir.dt.float32r
    BF = mybir.dt.bfloat16
    I16 = mybir.dt.int16
    EXP = mybir.ActivationFunctionType.Exp
    IDENT = mybir.ActivationFunctionType.Identity
    MUL = mybir.AluOpType.mult
    ADD = mybir.AluOpType.add

    nc = bacc.Bacc("TRN2", target_bir_lowering=False, debug=False, num_devices=1)

    xq = nc.dram_tensor("xq", [D, S], BF, kind="ExternalInput").ap()
    xk = nc.dram_tensor("xk", [D, S], BF, kind="ExternalInput").ap()
    xv = nc.dram_tensor("xv", [D, S], BF, kind="ExternalInput").ap()
    wq = nc.dram_tensor("wq", [128, DT * 256], BF, kind="ExternalInput").ap()
    wk = nc.dram_tensor("wk", [128, DT * 256], BF, kind="ExternalInput").ap()
    wv = nc.dram_tensor("wv", [128, DT * 260], BF, kind="ExternalInput").ap()
    wqb = nc.dram_tensor("wqb", [128, 2], FP, kind="ExternalInput").ap()
    wkb = nc.dram_tensor("wkb", [128, 2], FP, kind="ExternalInput").ap()
    wvb = nc.dram_tensor("wvb", [128, 260], BF, kind="ExternalInput").ap()
    wo = nc.dram_tensor("wo", [128, 2 * D], BF, kind="ExternalInput").ap()
    esb_d = nc.dram_tensor("esb", [8, 4 * 128], BF, kind="ExternalInput").ap()
    ones_d = nc.dram_tensor("ones", [1, 512], BF, kind="ExternalInput").ap()
    out = nc.dram_tensor("out_partial", [S, D], BF, kind="ExternalOutput").ap()

    with tile.TileContext(nc) as tc:
        with (
            tc.tile_pool(name="consts", bufs=1) as cpool,
            tc.tile_pool(name="resident", bufs=1) as rpool,
            tc.tile_pool(name="xin", bufs=24) as xpool,
            tc.tile_pool(name="exp", bufs=4) as epool,
            tc.tile_pool(name="outev", bufs=4) as opool,
            tc.tile_pool(name="stage", bufs=4) as spool,
            tc.tile_pool(name="psc", bufs=3, space="PSUM") as psc,
            tc.tile_pool(name="ppv", bufs=1, space="PSUM") as ppv,
        ):
            # ---- constants ----------------------------------------------
            ones = cpool.tile([1, 512], BF, tag="ones")
            nc.sync.dma_start(ones[:], ones_d[:])
            e_sb = cpool.tile([8, 4 * 128], BF, tag="esb")
            nc.sync.dma_start(e_sb[:], esb_d[:])
            wq_sb = cpool.tile([128, DT * 256], BF, tag="wq")
            nc.sync.dma_start(wq_sb[:], wq[:])
            wk_sb = cpool.tile([128, DT * 256], BF, tag="wk")
            nc.sync.dma_start(wk_sb[:], wk[:])
            wqb_sb = cpool.tile([128, 2], FP, tag="wqb")
            nc.sync.dma_start(wqb_sb[:], wqb[:])
            wkb_sb = cpool.tile([128, 2], FP, tag="wkb")
            nc.sync.dma_start(wkb_sb[:], wkb[:])

            # ---- x input tiles (k first, then v, then q) ----------------
            xk_t, xq_t, xv_t = [], [], []
            for dt in range(DT):
                t = xpool.tile([128, S], BF, tag="xin", name=f"xk{dt}")
                nc.sync.dma_start(t[:], xk[dt * 128 : (dt + 1) * 128, :])
                xk_t.append(t)
            wv_sb = cpool.tile([128, DT * 260], BF, tag="wv")
            nc.sync.dma_start(wv_sb[:], wv[:])
            wvb_sb = cpool.tile([128, 260], BF, tag="wvb")
            nc.sync.dma_start(wvb_sb[:], wvb[:])
            for dt in range(DT):
                t = xpool.tile([128, S], BF, tag="xin", name=f"xv{dt}")
                nc.sync.dma_start(t[:], xv[dt * 128 : (dt + 1) * 128, :])
                xv_t.append(t)
            for dt in range(DT):
                t = xpool.tile([128, S], BF, tag="xin", name=f"xq{dt}")
                nc.sync.dma_start(t[:], xq[dt * 128 : (dt + 1) * 128, :])
                xq_t.append(t)
            wo_sb = cpool.tile([128, 2 * D], BF, tag="wo")
            nc.sync.dma_start(wo_sb[:], wo[:])

            # ---- resident tiles -----------------------------------------
            khT = [rpool.tile([128, S], BF, tag=f"khT{j}", name=f"khT{j}") for j in range(2)]
            qhT = {
                (j, sc): rpool.tile([128, 512], BF, tag=f"qhT{j}{sc}", name=f"qhT{j}{sc}")
                for j in range(2)
                for sc in range(SC)
            }
            vh_t = [rpool.tile([128, 260], BF, tag=f"vh{tt}", name=f"vh{tt}") for tt in range(TT)]
            attnT = [rpool.tile([128, S], BF, tag=f"attnT{j}", name=f"attnT{j}") for j in range(2)]
            sums = [rpool.tile([8, 512], BF, tag=f"sums{j}", name=f"sums{j}") for j in range(2)]
            rtmp = [rpool.tile([8, 512], FP, tag=f"rtmp{j}", name=f"rtmp{j}") for j in range(2)]
            rtm2 = [rpool.tile([8, 512], FP, tag=f"rtm2{j}", name=f"rtm2{j}") for j in range(2)]
            recip = [rpool.tile([8, 512], BF, tag=f"recip{j}", name=f"recip{j}") for j in range(2)]

            # ---- PE warmup: HAM flips to 8/8 after ~3.4us of activity ----
            # garbage-valued source tile: no DMA dependency, result never read
            junk = cpool.tile([1, 512], BF, tag="junk")
            nc.vector.memset(junk[:], 1.0)
            wup = psc.tile([128, 1024], FP, tag="sc", name="wup")
            for _ in range(22):
                nc.tensor.matmul(
                    wup[:, 0:512], junk[0:1, 0:128], junk[0:1, :],
                    start=True, stop=True,
                )

            # ---- q/k projections (j-waved over 4 psum slots) ------------
            def qk_proj(x_t, w_sb, wb_sb, dst_j, dstq):
                for j in range(2):
                    pa = psc.tile([128, 1024], FP, tag="sc", name=f"ppa{j}")
                    pb2 = psc.tile([128, 1024], FP, tag="sc", name=f"ppb{j}")
                    pss = [pa[:, 0:512], pa[:, 512:1024], pb2[:, 0:512], pb2[:, 512:1024]]
                    for dt in range(DT):
                        for sc in range(SC):
                            nc.tensor.matmul(
                                pss[sc],
                                w_sb[:, dt * 256 + j * 128 : dt * 256 + (j + 1) * 128],
                                x_t[dt][:, sc * 512 : (sc + 1) * 512],
                                start=(dt == 0),
                                stop=(dt == DT - 1),
                            )
                    for sc in range(SC):
                        if dstq:
                            d = qhT[j, sc][:]
                        else:
                            d = dst_j[j][:, sc * 512 : (sc + 1) * 512]
                        if sc % 2 == 0:
                            nc.scalar.activation(
                                d, pss[sc], IDENT, bias=wb_sb[:, j : j + 1]
                            )
                        else:
                            nc.vector.tensor_scalar(
                                d, pss[sc], wb_sb[:, j : j + 1], None, ADD
                            )

            qk_proj(xk_t, wk_sb, wkb_sb, khT, False)

            # ---- v projection (two key-tiles per psum slot) -------------
            for tp in range(TT // 2):
                ps = psc.tile([128, 1024], FP, tag="sc", name=f"pv_{tp}")
                for u in range(2):
                    tt = tp * 2 + u
                    half = ps[:, u * 512 : u * 512 + 260]
                    for dt in range(DT):
                        nc.tensor.matmul(
                            half,
                            xv_t[dt][:, tt * 128 : (tt + 1) * 128],
                            wv_sb[:, dt * 260 : (dt + 1) * 260],
                            start=(dt == 0),
                            stop=(dt == DT - 1),
                        )
                    nc.vector.tensor_add(vh_t[tt][:], half, wvb_sb[:])

            qk_proj(xq_t, wq_sb, wqb_sb, None, True)

            # ---- attention ----------------------------------------------
            nc.vector.memset(sums[0][:], 1.0)
            nc.vector.memset(sums[1][:], 1.0)

            def normA(hp):
                with nc.allow_low_precision(reason="approx recip feeds softmax"):
                    nc.vector.tensor_copy(rtm2[hp][:], sums[hp][:])
                    nc.vector.reciprocal_approx_fast(rtmp[hp][:], rtm2[hp][:])
                    nc.vector.tensor_copy(recip[hp][:], rtmp[hp][:])

            _rbn = [0]

            def normB(hp, scs):
                rbt = None
                for i, sc in enumerate(scs):
                    if i % 2 == 0:
                        _rbn[0] += 1
                        rbt = psc.tile(
                            [128, 1024], FP, tag="sc", name=f"rb{_rbn[0]}"
                        )
                    rb = rbt[:, (i % 2) * 512 : (i % 2) * 512 + 512]
                    nc.tensor.matmul(
                        rb,
                        e_sb[:, sc * 128 : (sc + 1) * 128],
                        recip[hp][:],
                        start=True,
                        stop=True,
                    )
                    sl = attnT[hp][:, sc * 512 : (sc + 1) * 512]
                    nc.vector.tensor_mul(sl, sl, rb)

            pending = []

            for blk in range(2 * SC):
                hp, sc = blk // SC, blk % SC
                h0l, h1l = 2 * hp, 2 * hp + 1   # local head idx within core
                if True:
                    pvt = ppv.tile([EL, 1024], FP, tag="pv")
                    exq = []

                    def scores(tt):
                        ps = psc.tile([128, 1024], FP, tag="sc")
                        nc.tensor.matmul(
                            ps[:, 0:512],
                            khT[hp][0:64, tt * 128 : (tt + 1) * 128],
                            qhT[hp, sc][0:64, :],
                            start=True,
                            stop=True,
                        )
                        nc.tensor.matmul(
                            ps[:, 512:1024],
                            khT[hp][64:128, tt * 128 : (tt + 1) * 128],
                            qhT[hp, sc][64:128, :],
                            start=True,
                            stop=True,
                        )
                        if tt in DVE_TT:
                            zi = epool.tile([128, 1024], I16, tag="exD")
                            nc.vector.tensor_scalar(
                                zi[:], ps[:], float(EXP_A), float(EXP_B), MUL, ADD
                            )
                            exq.append((zi, True))
                        else:
                            ex = epool.tile([128, 1024], BF, tag="exA")
                            nc.scalar.activation(ex[:], ps[:], EXP, scale=0.125)
                            exq.append((ex, False))

                    def pv(tt):
                        ex, is_i16 = exq[tt]
                        e0 = ex[:, 0:512]
                        e1 = ex[:, 512:1024]
                        if is_i16:
                            e0 = e0.bitcast(mybir.dt.bfloat16)
                            e1 = e1.bitcast(mybir.dt.bfloat16)
                        nc.tensor.matmul(
                            pvt[0:EL, 0:512],
                            vh_t[tt][:, h0l * EL : h0l * EL + EL],
                            e0,
                            start=(tt == 0),
                            stop=(tt == TT - 1),
                        )
                        nc.tensor.matmul(
                            pvt[0:EL, 512:1024],
                            vh_t[tt][:, h1l * EL : h1l * EL + EL],
                            e1,
                            start=(tt == 0),
                            stop=(tt == TT - 1),
                        )

                    scores(0)
                    scores(1)
                    scores(2)
                    for fn in pending:
                        fn()
                    pending.clear()
                    for tt in range(TT - 3):
                        scores(tt + 3)
                        pv(tt)
                    pv(TT - 3)
                    pv(TT - 2)
                    pv(TT - 1)

                    st = spool.tile([EL, 1024], BF, tag="stage")
                    nc.scalar.copy(st[:, 0:512], pvt[0:EL, 0:512])
                    nc.vector.tensor_copy(st[:, 512:1024], pvt[0:EL, 512:1024])
                    nc.gpsimd.dma_start(
                        attnT[hp][0:64, sc * 512 : (sc + 1) * 512], st[0:E, 0:512]
                    )
                    nc.gpsimd.dma_start(
                        attnT[hp][64:128, sc * 512 : (sc + 1) * 512], st[0:E, 512:1024]
                    )
                    r0 = (h0l % 2) * 4 + sc
                    r1 = (h1l % 2) * 4 + sc
                    nc.gpsimd.dma_start(sums[hp][r0 : r0 + 1, :], st[E : E + 1, 0:512])
                    nc.gpsimd.dma_start(sums[hp][r1 : r1 + 1, :], st[E : E + 1, 512:1024])

                if blk == SC - 1:    # hp0 denominators all final
                    normA(0)
                    pending.append(lambda: normB(0, [0, 1]))
                elif blk == SC:
                    pending.append(lambda: normB(0, [2, 3]))
                elif blk == 2 * SC - 2:  # hp1 sc0/1/2 denominators final
                    normA(1)
                    pending.append(lambda: normB(1, [0, 1, 2]))
                elif blk == 2 * SC - 1:
                    # keep the PE warm through the final recip chain
                    fil = psc.tile([128, 1024], FP, tag="sc", name="fil")
                    for _ in range(10):
                        nc.tensor.matmul(
                            fil[:, 0:512], junk[0:1, 0:128], junk[0:1, :],
                            start=True, stop=True,
                        )
                    normA(1)
                    normB(1, [3])

            # ---- output projection --------------------------------------
            for st in range(TT):
                ps = psc.tile([128, 1024], FP, tag="sc", name=f"op{st}")
                for nh in range(2):
                    for j in range(2):
                        nc.tensor.matmul(
                            ps[:, nh * 512 : (nh + 1) * 512],
                            attnT[j][:, st * 128 : (st + 1) * 128],
                            wo_sb[:, j * D + nh * 512 : j * D + (nh + 1) * 512],
                            start=(j == 0),
                            stop=(j == 1),
                        )
                ot = opool.tile([128, 1024], BF, tag="outev")
                nc.vector.tensor_copy(ot[:, 0:512], ps[:, 0:512])
                nc.scalar.copy(ot[:, 512:1024], ps[:, 512:1024])
                nc.sync.dma_start(out[st * 128 : (st + 1) * 128, :], ot[:])

    nc.compile()
    return nc


def _get_nc():
    global _NC
    if _NC is None:
        _NC = _build()
    return _NC


def _in_maps(q, k, v, Wq, bq, Wk, bk, Wv, bv, Wo, bo):
    import ml_dtypes

    f32 = np.float32
    bf16 = ml_dtypes.bfloat16
    maps = []
    for c in range(N_CORES):
        b, hg = c // HG, c % HG
        hs = slice(hg * HG, (hg + 1) * HG)  # this core's 4 heads

        # wq/wk staged as [128, dt*256]: row p, block dt = original row dt*128+p
        wq_flat = np.transpose(Wq[hs], (1, 0, 2)).reshape(D, HG * E)
        wk_flat = np.transpose(Wk[hs], (1, 0, 2)).reshape(D, HG * E)
        wq_h = wq_flat.reshape(DT, 128, 256).transpose(1, 0, 2).reshape(128, DT * 256)
        wk_h = wk_flat.reshape(DT, 128, 256).transpose(1, 0, 2).reshape(128, DT * 256)
        # biases per j-block: [128, 2]
        wqb_h = bq[hs].reshape(2, 128).T.copy()
        wkb_h = bk[hs].reshape(2, 128).T.copy()
        # wv staged as [128, dt*260] + bias row [1, 260] (with ones column)
        wv_flat = np.zeros((D, HG * EL), f32)
        wvb_h = np.zeros((1, HG * EL), f32)
        for hl in range(HG):
            wv_flat[:, hl * EL : hl * EL + E] = Wv[hg * HG + hl]
            wvb_h[0, hl * EL : hl * EL + E] = bv[hg * HG + hl]
            wvb_h[0, hl * EL + E] = 1.0  # ones column of vh'
        wv_h = wv_flat.reshape(DT, 128, 260).transpose(1, 0, 2).reshape(128, DT * 260)
        # wo staged as [128, 2*D]: block j = rows of Wo for head pair j
        wo_h = np.zeros((128, 2 * D), f32)
        for j in range(2):
            wo_h[:, j * D : (j + 1) * D] = Wo[
                (hg * HG + 2 * j) * E : (hg * HG + 2 * j + 2) * E, :
            ]
        maps.append(
            {
                "xq": np.ascontiguousarray(q[b].T).astype(bf16),
                "xk": np.ascontiguousarray(k[b].T).astype(bf16),
                "xv": np.ascontiguousarray(v[b].T).astype(bf16),
                "wq": wq_h.astype(bf16),
                "wk": wk_h.astype(bf16),
                "wv": wv_h.astype(bf16),
                "wqb": wqb_h.astype(f32),
                "wkb": wkb_h.astype(f32),
                "wvb": np.tile(wvb_h, (128, 1)).astype(bf16),
                "wo": wo_h.astype(bf16),
                "esb": _ESB.astype(bf16),
                "ones": _ONES.astype(bf16),
            }
        )
    return maps


def _run(inputs, trace=False):
    from concourse.bass_utils import run_bass_kernel_spmd

    nc = _get_nc()
    maps = _in_maps(**inputs)
    res = run_bass_kernel_spmd(nc, maps, list(range(N_CORES)), trace=trace)
    bo = np.asarray(inputs["bo"], np.float32)
    out = np.zeros((B, S, D), np.float32)
    for b in range(B):
        acc = np.zeros((S, D), np.float32)
        for hg in range(HG):
            acc += res.results[b * HG + hg]["out_partial"].astype(np.float32)
        out[b] = acc + bo[None, :]
    return out, res.exec_time_ns


def kernel(**inputs):
    out, _ = _run(inputs, trace=False)
    return out


def kernel_traced(**inputs):
    return _run(inputs, trace=True)
